# revision 7
# baseline (speedup 1.0000x reference)
"""Trainium2 Bass kernel for nn_DiffusionGraphConv_78374563217429.

Math reformulation (exact algebra):
  reference out = concat_m(x_m) @ W  with  xs = [x0, A0 x0, 2 A0^2 x0 - x0,
                                                 A1 x0, 2 A1^2 x0 - x0]
  Since everything is linear, push W through the recurrence:
      out = x0 @ Wd + sum_s A_s @ (x0 @ W1s + A_s @ (x0 @ 2 W2s))
  with Wd = W0 - W20 - W21.  This shrinks each SpMM application from 128
  features to 64 and removes the big final [B*N,640]@[640,64] matmul.

Implementation: the COO supports are densified host-side (static graph
preprocessing) into fp8-e4m3 [4096,4096] matrices (scaled x16 into the
healthy e4m3 range) laid out in matmul-ready panels; on each core the
recurrence runs as dense TensorE matmuls in DoubleRow fp8 perf mode
(256-deep contraction per instruction) with fp32 PSUM accumulation.
Power-of-2 scales keep every fp8 operand near unit RMS:
  u_s  = x0 @ (2 W2s)          (fp8)
  wt_s = x0 @ (16 W1s)         (fp16)
  w_s  = A8_s @ u_s + wt_s     (fp8;  = 16 w_s_true)
  out  = (A8_s @ w_s) * 2^-8 + init   (init = x0 @ Wd, fp16)
Host emulation of this exact pipeline gives rel err 4.5e-3 (gate 2e-2).

Sharding: data-parallel over batch, 4 batch items per core x 8 cores;
supports/weights replicated.
"""

import os
import sys

import numpy as np

# ---------------------------------------------------------------- constants
P = 128          # partitions
N = 4096         # nodes
NM = 32          # output-node chunks (N / P)
NK = 32          # contraction-node chunks (N / P)
BC = 4           # batch items per core
FREE = BC * 64   # matmul moving free dim for SpMM passes (4 batches x 64 feat)
PW = 320         # P-phase Wcat columns: [u0 | u1 | wt0 | wt1 | init] x 64
NCORES = 8

_COMPILED = None     # cached (nc, ) across kernel() calls
LAST_RESULTS = None  # BassKernelResults of the most recent run (for test.py)


def _import_concourse():
    try:
        import concourse.bass  # noqa: F401
    except ImportError:
        for p in ("/opt/trn_rl_repo", "/root/.axon_site/_ro/trn_rl_repo"):
            if os.path.isdir(p) and p not in sys.path:
                sys.path.insert(0, p)
        import concourse.bass  # noqa: F401
    # bass_utils imports antenv.axon_hooks when tracing is requested; some
    # images lack that module — stub it so BASS_TRACE never crashes the run.
    try:
        import antenv.axon_hooks  # noqa: F401
    except ImportError:
        import types
        mod = types.ModuleType("antenv.axon_hooks")
        mod.get_axon_ntff_profile_hook = lambda: None
        mod.set_axon_ntff_profile_hook = lambda h: None
        sys.modules["antenv.axon_hooks"] = mod


def _build_module():
    """Trace the Bass/Tile module (identical SPMD program for all 8 cores)."""
    import concourse.mybir as mybir
    from concourse import bacc
    from concourse.tile import TileContext

    f8 = mybir.dt.float8e4
    f16 = mybir.dt.float16
    f32 = mybir.dt.float32
    DR = mybir.MatmulPerfMode.DoubleRow
    MULT = mybir.AluOpType.mult
    ADD = mybir.AluOpType.add

    nc = bacc.Bacc("TRN2", target_bir_lowering=False, debug=False,
                   num_devices=NCORES)

    at0 = nc.dram_tensor("at0", [NM, P, NK, P], f8, kind="ExternalInput").ap()
    at1 = nc.dram_tensor("at1", [NM, P, NK, P], f8, kind="ExternalInput").ap()
    x0t = nc.dram_tensor("x0t", [BC, P, N], f16, kind="ExternalInput").ap()
    wcat = nc.dram_tensor("wcat", [P, PW], f16, kind="ExternalInput").ap()
    outd = nc.dram_tensor("out", [P, NM * FREE], f16, kind="ExternalOutput").ap()

    ats = (at0, at1)

    with TileContext(nc) as tc:
        with (
            tc.tile_pool(name="singles", bufs=1) as singles,
            tc.tile_pool(name="trans", bufs=6) as trans,
        ):
            wcat_sb = singles.tile([P, PW], f16, name="wcat_sb")
            nc.sync.dma_start(out=wcat_sb, in_=wcat)

            # ---- PE warmup: HAM clock-gate starts at 1.2 GHz and only
            # releases after ~3.4us of sustained PE activity.  Fill the
            # DMA-load window with dummy matmuls so the real work runs
            # at 2.4 GHz from the first instruction.
            wlhs = singles.tile([P, P], f16, name="wlhs")
            wrhs = singles.tile([P, 512], f16, name="wrhs")
            nc.vector.memset(wlhs, 0.0)
            nc.vector.memset(wrhs, 0.0)

            # persistent SBUF buffers
            # bigp8 sections: 0=u0, 1=u1 (fp8, SpMM rhs; layout [p,s,m,b*64+f])
            # bigp16 sections: 0=wt0, 1=wt1, 2=init (fp16)
            bigp8 = singles.tile([P, 2, NM, FREE], f8, name="bigp8")
            bigp16 = singles.tile([P, 3, NM, FREE], f16, name="bigp16")
            w0_sb = singles.tile([P, NK, FREE], f8, name="w0_sb")
            w1_sb = singles.tile([P, NK, FREE], f8, name="w1_sb")
            out_sb = singles.tile([P, NM, FREE], f16, name="out_sb")

            # all four batches' feature-major x0 tiles, loaded upfront in
            # quarter-tiles spread over four HWDGE rings
            xts = singles.tile([P, BC, N], f16, name="xts")
            xrings = (nc.sync, nc.scalar)
            qn = 0
            for b in range(BC):
                for h in range(4):
                    xrings[qn % 2].dma_start(
                        out=xts[:, b, h * (N // 4):(h + 1) * (N // 4)],
                        in_=x0t[b, :, h * (N // 4):(h + 1) * (N // 4)])
                    qn += 1

            # ---------------- P phase: P_b = x0_b @ Wcat ----------------
            # Wcat col sections: [0:64]=2*W20 (u0), [64:128]=2*W21 (u1),
            # [128:192]=16*W10 (wt0), [192:256]=16*W11 (wt1),
            # [256:320]=Wd (init)
            pp_cm = tc.tile_pool(name="pp", bufs=2, space="PSUM")
            pp = pp_cm.__enter__()
            wps = pp.tile([P, 4, 512], f32, tag="pp_ps", name="warm_ps")
            for _ in range(20):
                nc.tensor.matmul(wps[:, 0, :FREE], wlhs, wrhs[:, :FREE],
                                 start=True, stop=True)

            # PSUM->SBUF evacuation engines, rotated per copy so no single
            # engine's queue paces the P phase
            def _dve(out, in_):
                return nc.vector.tensor_copy(out=out, in_=in_)

            def _act(out, in_):
                return nc.scalar.copy(out=out, in_=in_)

            engs = (_dve, _act)
            ei = 0
            for b in range(BC):
                for mq in range(NM // 4):
                    ps = pp.tile([P, 4, 512], f32, tag="pp_ps", name="pp_ps")
                    for mi in range(4):
                        m = mq * 4 + mi
                        nc.tensor.matmul(
                            ps[:, mi, :PW],
                            xts[:, b, m * P:(m + 1) * P],
                            wcat_sb,
                            start=True, stop=True,
                        )
                    engs[ei % 2](
                        out=bigp8[:, :, mq * 4:(mq + 1) * 4, b * 64:(b + 1) * 64],
                        in_=ps[:, :, 0:128].rearrange(
                            "p m (s f) -> p s m f", f=64),
                    )
                    engs[(ei + 1) % 2](
                        out=bigp16[:, :, mq * 4:(mq + 1) * 4, b * 64:(b + 1) * 64],
                        in_=ps[:, :, 128:PW].rearrange(
                            "p m (s f) -> p s m f", f=64),
                    )
                    ei += 2
            pp_cm.__exit__(None, None, None)

            # ---------------- SpMM passes (fp8 DoubleRow) ----------------
            sp_cm = tc.tile_pool(name="sp", bufs=2, space="PSUM")
            sp = sp_cm.__enter__()

            def spmm_pass(at_ap, rhs_fn, post_fn):
                for m in range(NM):
                    panel = trans.tile([P, NK, P], f8, tag="big8k", name="panel")
                    # alternate HWDGE rings (SP / ACT) so panel loads stream
                    # on both queues instead of one FIFO
                    dma_eng = nc.sync if m % 2 == 0 else nc.scalar
                    dma_eng.dma_start(out=panel, in_=at_ap[m])
                    ps = sp.tile([P, FREE], f32, name="sp_ps")
                    for kp in range(NK // 2):
                        nc.tensor.matmul(
                            ps,
                            panel[:, 2 * kp:2 * kp + 2, :],
                            rhs_fn(kp),
                            start=(kp == 0), stop=(kp == NK // 2 - 1),
                            perf_mode=DR,
                        )
                    post_fn(m, ps)

            # v0 = A8_0 @ u0 ;  w0 = v0 + wt0
            spmm_pass(
                ats[0],
                lambda kp: bigp8[:, 0, 2 * kp:2 * kp + 2, :],
                lambda m, ps: nc.vector.tensor_add(
                    out=w0_sb[:, m, :], in0=ps, in1=bigp16[:, 0, m, :]),
            )
            # v1 = A8_1 @ u1 ;  w1 = v1 + wt1
            spmm_pass(
                ats[1],
                lambda kp: bigp8[:, 1, 2 * kp:2 * kp + 2, :],
                lambda m, ps: nc.vector.tensor_add(
                    out=w1_sb[:, m, :], in0=ps, in1=bigp16[:, 1, m, :]),
            )
            # t0 = A8_0 @ w0 ;  out = t0 * 2^-8 + init
            spmm_pass(
                ats[0],
                lambda kp: w0_sb[:, 2 * kp:2 * kp + 2, :],
                lambda m, ps: nc.vector.scalar_tensor_tensor(
                    out=out_sb[:, m, :], in0=ps, scalar=2.0 ** -8,
                    in1=bigp16[:, 2, m, :], op0=MULT, op1=ADD),
            )
            # t1 = A8_1 @ w1 ;  out += t1 * 2^-8 ; stream result out per chunk
            outd_v = outd.rearrange("p (m f) -> p m f", f=FREE)

            def _t1_post(m, ps):
                nc.vector.scalar_tensor_tensor(
                    out=out_sb[:, m, :], in0=ps, scalar=2.0 ** -8,
                    in1=out_sb[:, m, :], op0=MULT, op1=ADD)
                # each store rides the ring opposite its chunk's panel load
                store_eng = nc.scalar if m % 2 == 0 else nc.sync
                store_eng.dma_start(out=outd_v[:, m, :], in_=out_sb[:, m, :])

            spmm_pass(ats[1], lambda kp: w1_sb[:, 2 * kp:2 * kp + 2, :],
                      _t1_post)
            sp_cm.__exit__(None, None, None)

    nc.compile()
    return nc


def _get_compiled():
    global _COMPILED
    if _COMPILED is None:
        _import_concourse()
        _COMPILED = _build_module()
    return _COMPILED


def _f8_dtype():
    import ml_dtypes
    if hasattr(ml_dtypes, "float8_e4m3"):
        return ml_dtypes.float8_e4m3
    return ml_dtypes.float8_e4m3fn


def _densify_panels(rows, cols, vals):
    """COO -> dense fp8 (x16 scaled) panels at[m, p, kc, j] = 16*A[m*128+j, kc*128+p]."""
    A = np.zeros((N, N), np.float32)
    np.add.at(A, (np.asarray(rows), np.asarray(cols)), np.asarray(vals))
    at = (16.0 * A).reshape(NM, P, NK, P).transpose(0, 3, 2, 1)
    return np.ascontiguousarray(at).astype(_f8_dtype())


def kernel(inputs, state, rows0, cols0, vals0, rows1, cols1, vals1,
           weight, biases, output_size):
    global LAST_RESULTS
    _import_concourse()
    from concourse.bass_utils import run_bass_kernel_spmd

    inputs = np.asarray(inputs, dtype=np.float32)
    state = np.asarray(state, dtype=np.float32)
    weight = np.asarray(weight, dtype=np.float32)
    biases = np.asarray(biases, dtype=np.float32)
    B = inputs.shape[0]
    assert B == NCORES * BC

    # ---- host prep: static graph/weight preprocessing + layout ----
    at0 = _densify_panels(rows0, cols0, vals0)
    at1 = _densify_panels(rows1, cols1, vals1)

    W = weight.reshape(P, 5, 64)  # [feat, matrix, out]
    W0, W10, W20, W11, W21 = (W[:, m, :] for m in range(5))
    wcat = np.concatenate(
        [2.0 * W20, 2.0 * W21, 16.0 * W10, 16.0 * W11, W0 - W20 - W21], axis=1
    ).astype(np.float16)
    wcat = np.ascontiguousarray(wcat)

    # feat-major x0 per batch: x0t[b, f, n]
    xin = inputs.reshape(B, N, 64)
    xst = state.reshape(B, N, 64)
    x0t = np.empty((B, P, N), np.float16)
    x0t[:, :64, :] = xin.transpose(0, 2, 1)
    x0t[:, 64:, :] = xst.transpose(0, 2, 1)

    nc = _get_compiled()
    in_maps = [
        {
            "at0": at0,
            "at1": at1,
            "wcat": wcat,
            "x0t": np.ascontiguousarray(x0t[c * BC:(c + 1) * BC]),
        }
        for c in range(NCORES)
    ]
    # The axon terminal occasionally reports NRT_EXEC_UNIT_UNRECOVERABLE on
    # the first execution of a freshly compiled NEFF; a reload retry succeeds.
    last_exc = None
    for _attempt in range(3):
        try:
            res = run_bass_kernel_spmd(nc, in_maps, core_ids=list(range(NCORES)))
            break
        except Exception as e:  # noqa: BLE001
            last_exc = e
            import time
            time.sleep(5.0)
    else:
        raise last_exc
    LAST_RESULTS = res

    out = np.empty((B, N * 64), np.float32)
    for c in range(NCORES):
        r = np.asarray(res.results[c]["out"]).astype(np.float32)  # [P, NM*FREE]
        # r[p, m*256 + bi*64 + f] = out[bi, m*128+p, f]
        out[c * BC:(c + 1) * BC] = (
            r.reshape(P, NM, BC, 64).transpose(2, 1, 0, 3).reshape(BC, N * 64)
        )
    # biases are all zeros in this problem spec, but honor them anyway
    if np.any(biases):
        out += np.tile(biases, N)[None, :]
    return out


# revision 12
# speedup vs baseline: 1.0443x; 1.0443x over previous
"""Trainium2 Bass kernel for nn_DiffusionGraphConv_78374563217429.

Math reformulation (exact algebra):
  reference out = concat_m(x_m) @ W  with  xs = [x0, A0 x0, 2 A0^2 x0 - x0,
                                                 A1 x0, 2 A1^2 x0 - x0]
  Since everything is linear, push W through the recurrence:
      out = x0 @ Wd + sum_s A_s @ (x0 @ W1s + A_s @ (x0 @ 2 W2s))
  with Wd = W0 - W20 - W21.  This shrinks each SpMM application from 128
  features to 64 and removes the big final [B*N,640]@[640,64] matmul.

Implementation: the COO supports are densified host-side (static graph
preprocessing) into fp8-e4m3 [4096,4096] matrices (scaled x16 into the
healthy e4m3 range) laid out in matmul-ready panels; on each core the
recurrence runs as dense TensorE matmuls in DoubleRow fp8 perf mode
(256-deep contraction per instruction) with fp32 PSUM accumulation.
Power-of-2 scales keep every fp8 operand near unit RMS:
  u_s  = x0 @ (2 W2s)          (fp8)
  wt_s = x0 @ (16 W1s)         (fp16)
  w_s  = A8_s @ u_s + wt_s     (fp8;  = 16 w_s_true)
  out  = (A8_s @ w_s) * 2^-8 + init   (init = x0 @ Wd, fp16)
Host emulation of this exact pipeline gives rel err 4.5e-3 (gate 2e-2).

Sharding: data-parallel over batch, 4 batch items per core x 8 cores;
supports/weights replicated.
"""

import os
import sys

import numpy as np

# ---------------------------------------------------------------- constants
P = 128          # partitions
N = 4096         # nodes
NM = 32          # output-node chunks (N / P)
NK = 32          # contraction-node chunks (N / P)
BC = 4           # batch items per core
FREE = BC * 64   # matmul moving free dim for SpMM passes (4 batches x 64 feat)
PW = 320         # P-phase Wcat columns: [u0 | u1 | wt0 | wt1 | init] x 64
NCORES = 8

_COMPILED = None     # cached (nc, ) across kernel() calls
LAST_RESULTS = None  # BassKernelResults of the most recent run (for test.py)


def _import_concourse():
    try:
        import concourse.bass  # noqa: F401
    except ImportError:
        for p in ("/opt/trn_rl_repo", "/root/.axon_site/_ro/trn_rl_repo"):
            if os.path.isdir(p) and p not in sys.path:
                sys.path.insert(0, p)
        import concourse.bass  # noqa: F401
    # bass_utils imports antenv.axon_hooks when tracing is requested; some
    # images lack that module — stub it so BASS_TRACE never crashes the run.
    try:
        import antenv.axon_hooks  # noqa: F401
    except ImportError:
        import types
        mod = types.ModuleType("antenv.axon_hooks")
        mod.get_axon_ntff_profile_hook = lambda: None
        mod.set_axon_ntff_profile_hook = lambda h: None
        sys.modules["antenv.axon_hooks"] = mod


def _build_module():
    """Trace the Bass/Tile module (identical SPMD program for all 8 cores)."""
    import concourse.mybir as mybir
    from concourse import bacc
    from concourse.tile import TileContext

    f8 = mybir.dt.float8e4
    f16 = mybir.dt.float16
    f32 = mybir.dt.float32
    DR = mybir.MatmulPerfMode.DoubleRow
    MULT = mybir.AluOpType.mult
    ADD = mybir.AluOpType.add

    nc = bacc.Bacc("TRN2", target_bir_lowering=False, debug=False,
                   num_devices=NCORES)

    at0 = nc.dram_tensor("at0", [NM, P, NK, P], f8, kind="ExternalInput").ap()
    at1 = nc.dram_tensor("at1", [NM, P, NK, P], f8, kind="ExternalInput").ap()
    x0t = nc.dram_tensor("x0t", [BC, P, N], f16, kind="ExternalInput").ap()
    wcat = nc.dram_tensor("wcat", [P, PW], f16, kind="ExternalInput").ap()
    outd = nc.dram_tensor("out", [P, NM * FREE], f16, kind="ExternalOutput").ap()

    ats = (at0, at1)

    NCACHE = 16  # A1 panel chunks kept resident between pass 2 and pass 4

    with TileContext(nc) as tc:
        with (
            tc.tile_pool(name="singles", bufs=1) as singles,
            tc.tile_pool(name="trans", bufs=6) as trans,
        ):
            wcat_sb = singles.tile([P, PW], f16, name="wcat_sb")
            nc.sync.dma_start(out=wcat_sb, in_=wcat)

            # ---- PE warmup: HAM clock-gate starts at 1.2 GHz and only
            # releases after ~3.4us of sustained PE activity.  Fill the
            # DMA-load window with dummy matmuls so the real work runs
            # at 2.4 GHz from the first instruction.
            wlhs = singles.tile([P, P], f16, name="wlhs")
            wrhs = singles.tile([P, 512], f16, name="wrhs")
            nc.vector.memset(wlhs, 0.0)
            nc.vector.memset(wrhs, 0.0)

            # persistent SBUF buffers
            # bigp8 sections: 0=u0, 1=u1 (fp8, SpMM rhs; layout [p,s,m,b*64+f])
            # bigp16 sections: 0=wt0, 1=wt1, 2=init (fp16)
            bigp8 = singles.tile([P, 2, NM, FREE], f8, name="bigp8")
            bigp16 = singles.tile([P, 3, NM, FREE], f16, name="bigp16")
            w0_sb = singles.tile([P, NK, FREE], f8, name="w0_sb")
            w1_sb = singles.tile([P, NK, FREE], f8, name="w1_sb")
            out_sb = singles.tile([P, NM, FREE], f16, name="out_sb")

            # all four batches' feature-major x0 tiles, loaded upfront in
            # quarter-tiles alternating over both HWDGE rings; the buffer is
            # released after the P phase to make room for the A1 panel cache
            xp_cm = tc.tile_pool(name="xp", bufs=1)
            xp = xp_cm.__enter__()
            xts = xp.tile([P, BC, N], f16, name="xts")
            xrings = (nc.sync, nc.scalar)
            qn = 0
            for b in range(BC):
                for h in range(4):
                    xrings[qn % 2].dma_start(
                        out=xts[:, b, h * (N // 4):(h + 1) * (N // 4)],
                        in_=x0t[b, :, h * (N // 4):(h + 1) * (N // 4)])
                    qn += 1

            # ---------------- P phase: P_b = x0_b @ Wcat ----------------
            # Wcat col sections: [0:64]=2*W20 (u0), [64:128]=2*W21 (u1),
            # [128:192]=16*W10 (wt0), [192:256]=16*W11 (wt1),
            # [256:320]=Wd (init)
            pp_cm = tc.tile_pool(name="pp", bufs=4, space="PSUM")
            pp = pp_cm.__enter__()
            wps = pp.tile([P, 2, 512], f32, tag="pp_ps", name="warm_ps")
            for _ in range(20):
                nc.tensor.matmul(wps[:, 0, :FREE], wlhs, wrhs[:, :FREE],
                                 start=True, stop=True)

            # PSUM->SBUF evacuation engines, rotated per copy so no single
            # engine's queue paces the P phase
            def _dve(out, in_):
                return nc.vector.tensor_copy(out=out, in_=in_)

            def _act(out, in_):
                return nc.scalar.copy(out=out, in_=in_)

            engs = (_dve, _act)
            ei = 0
            for b in range(BC):
                for mg in range(NM // 2):
                    ps = pp.tile([P, 2, 512], f32, tag="pp_ps", name="pp_ps")
                    for mi in range(2):
                        m = mg * 2 + mi
                        nc.tensor.matmul(
                            ps[:, mi, :PW],
                            xts[:, b, m * P:(m + 1) * P],
                            wcat_sb,
                            start=True, stop=True,
                        )
                    engs[ei % 2](
                        out=bigp8[:, :, mg * 2:(mg + 1) * 2, b * 64:(b + 1) * 64],
                        in_=ps[:, :, 0:128].rearrange(
                            "p m (s f) -> p s m f", f=64),
                    )
                    engs[(ei + 1) % 2](
                        out=bigp16[:, :, mg * 2:(mg + 1) * 2, b * 64:(b + 1) * 64],
                        in_=ps[:, :, 128:PW].rearrange(
                            "p m (s f) -> p s m f", f=64),
                    )
                    ei += 1
            pp_cm.__exit__(None, None, None)
            xp_cm.__exit__(None, None, None)

            # ---------------- SpMM passes (fp8 DoubleRow) ----------------
            sp_cm = tc.tile_pool(name="sp", bufs=2, space="PSUM")
            sp = sp_cm.__enter__()
            cache_cm = tc.tile_pool(name="a1cache", bufs=1)
            a1cache = cache_cm.__enter__()
            cached_panels = {}

            def spmm_pass(at_ap, rhs_fn, post_fn, fill_cache=False,
                          use_cache=False):
                for m in range(NM):
                    if use_cache and m in cached_panels:
                        panel = cached_panels[m]
                    else:
                        if fill_cache and m < NCACHE:
                            panel = a1cache.tile([P, NK, P], f8,
                                                 name=f"a1c{m}")
                            cached_panels[m] = panel
                        else:
                            panel = trans.tile([P, NK, P], f8, tag="big8k",
                                               name="panel")
                        # alternate HWDGE rings (SP / ACT) so panel loads
                        # stream on both queues instead of one FIFO
                        dma_eng = nc.sync if m % 2 == 0 else nc.scalar
                        dma_eng.dma_start(out=panel, in_=at_ap[m])
                    ps = sp.tile([P, FREE], f32, name="sp_ps")
                    for kp in range(NK // 2):
                        nc.tensor.matmul(
                            ps,
                            panel[:, 2 * kp:2 * kp + 2, :],
                            rhs_fn(kp),
                            start=(kp == 0), stop=(kp == NK // 2 - 1),
                            perf_mode=DR,
                        )
                    post_fn(m, ps)

            # v0 = A8_0 @ u0 ;  w0 = v0 + wt0
            spmm_pass(
                ats[0],
                lambda kp: bigp8[:, 0, 2 * kp:2 * kp + 2, :],
                lambda m, ps: nc.vector.tensor_add(
                    out=w0_sb[:, m, :], in0=ps, in1=bigp16[:, 0, m, :]),
            )
            # v1 = A8_1 @ u1 ;  w1 = v1 + wt1  (fills the A1 panel cache)
            spmm_pass(
                ats[1],
                lambda kp: bigp8[:, 1, 2 * kp:2 * kp + 2, :],
                lambda m, ps: nc.vector.tensor_add(
                    out=w1_sb[:, m, :], in0=ps, in1=bigp16[:, 1, m, :]),
                fill_cache=True,
            )
            # t0 = A8_0 @ w0 ;  out = t0 * 2^-8 + init
            spmm_pass(
                ats[0],
                lambda kp: w0_sb[:, 2 * kp:2 * kp + 2, :],
                lambda m, ps: nc.vector.scalar_tensor_tensor(
                    out=out_sb[:, m, :], in0=ps, scalar=2.0 ** -8,
                    in1=bigp16[:, 2, m, :], op0=MULT, op1=ADD),
            )
            # t1 = A8_1 @ w1 ;  out += t1 * 2^-8 ; stream result out per chunk
            outd_v = outd.rearrange("p (m f) -> p m f", f=FREE)

            def _t1_post(m, ps):
                nc.vector.scalar_tensor_tensor(
                    out=out_sb[:, m, :], in0=ps, scalar=2.0 ** -8,
                    in1=out_sb[:, m, :], op0=MULT, op1=ADD)
                # each store rides the ring opposite its chunk's panel load
                store_eng = nc.scalar if m % 2 == 0 else nc.sync
                store_eng.dma_start(out=outd_v[:, m, :], in_=out_sb[:, m, :])

            spmm_pass(ats[1], lambda kp: w1_sb[:, 2 * kp:2 * kp + 2, :],
                      _t1_post, use_cache=True)
            cache_cm.__exit__(None, None, None)
            sp_cm.__exit__(None, None, None)

    nc.compile()
    return nc


def _get_compiled():
    global _COMPILED
    if _COMPILED is None:
        _import_concourse()
        _COMPILED = _build_module()
    return _COMPILED


def _f8_dtype():
    import ml_dtypes
    if hasattr(ml_dtypes, "float8_e4m3"):
        return ml_dtypes.float8_e4m3
    return ml_dtypes.float8_e4m3fn


def _densify_panels(rows, cols, vals):
    """COO -> dense fp8 (x16 scaled) panels at[m, p, kc, j] = 16*A[m*128+j, kc*128+p]."""
    A = np.zeros((N, N), np.float32)
    np.add.at(A, (np.asarray(rows), np.asarray(cols)), np.asarray(vals))
    at = (16.0 * A).reshape(NM, P, NK, P).transpose(0, 3, 2, 1)
    return np.ascontiguousarray(at).astype(_f8_dtype())


def kernel(inputs, state, rows0, cols0, vals0, rows1, cols1, vals1,
           weight, biases, output_size):
    global LAST_RESULTS
    _import_concourse()
    from concourse.bass_utils import run_bass_kernel_spmd

    inputs = np.asarray(inputs, dtype=np.float32)
    state = np.asarray(state, dtype=np.float32)
    weight = np.asarray(weight, dtype=np.float32)
    biases = np.asarray(biases, dtype=np.float32)
    B = inputs.shape[0]
    assert B == NCORES * BC

    # ---- host prep: static graph/weight preprocessing + layout ----
    at0 = _densify_panels(rows0, cols0, vals0)
    at1 = _densify_panels(rows1, cols1, vals1)

    W = weight.reshape(P, 5, 64)  # [feat, matrix, out]
    W0, W10, W20, W11, W21 = (W[:, m, :] for m in range(5))
    wcat = np.concatenate(
        [2.0 * W20, 2.0 * W21, 16.0 * W10, 16.0 * W11, W0 - W20 - W21], axis=1
    ).astype(np.float16)
    wcat = np.ascontiguousarray(wcat)

    # feat-major x0 per batch: x0t[b, f, n]
    xin = inputs.reshape(B, N, 64)
    xst = state.reshape(B, N, 64)
    x0t = np.empty((B, P, N), np.float16)
    x0t[:, :64, :] = xin.transpose(0, 2, 1)
    x0t[:, 64:, :] = xst.transpose(0, 2, 1)

    nc = _get_compiled()
    in_maps = [
        {
            "at0": at0,
            "at1": at1,
            "wcat": wcat,
            "x0t": np.ascontiguousarray(x0t[c * BC:(c + 1) * BC]),
        }
        for c in range(NCORES)
    ]
    # The axon terminal occasionally reports NRT_EXEC_UNIT_UNRECOVERABLE on
    # the first execution of a freshly compiled NEFF; a reload retry succeeds.
    last_exc = None
    for _attempt in range(3):
        try:
            res = run_bass_kernel_spmd(nc, in_maps, core_ids=list(range(NCORES)))
            break
        except Exception as e:  # noqa: BLE001
            last_exc = e
            import time
            time.sleep(5.0)
    else:
        raise last_exc
    LAST_RESULTS = res

    out = np.empty((B, N * 64), np.float32)
    for c in range(NCORES):
        r = np.asarray(res.results[c]["out"]).astype(np.float32)  # [P, NM*FREE]
        # r[p, m*256 + bi*64 + f] = out[bi, m*128+p, f]
        out[c * BC:(c + 1) * BC] = (
            r.reshape(P, NM, BC, 64).transpose(2, 1, 0, 3).reshape(BC, N * 64)
        )
    # biases are all zeros in this problem spec, but honor them anyway
    if np.any(biases):
        out += np.tile(biases, N)[None, :]
    return out
